# revision 23
# baseline (speedup 1.0000x reference)
"""Trainium2 Bass kernel for DecoderRNNWithAttention (teacher-forced LSTM decoder).

Key mathematical simplification: the attention block is an exact no-op.
The encoder output has a single spatial position, so softmax over that
axis is exactly 1.0 and context == features, independent of h. Hence:
  - the enc/dec/full attention projections never affect the output;
  - the input-side gate contributions Gx = X @ W_ih.T + (b_ih + b_hh)
    split into a word part (per timestep) and a features part that is
    the SAME for every t: F = features @ W_ih[:, E:].T + bias. F is
    computed on the host (52 MFLOP) and injected into each PSUM
    accumulation through a K=16 indicator matmul (rhs = tiled identity),
    so the device only runs the word matmul at N=400;
  - the serial recurrence is gates_t = Gx_t + h_t @ W_hh.T plus the
    LSTM elementwise cell; logits_t = h_{t+1} @ fcn_W.T + fcn_b.

Performance structure (per core, data-parallel over batch, no collectives):
  - phase 2 is weight-load bound (256 [128x128] W_hh tiles per step,
    N=16 free): W_hh is stored fp8e4m3 so the fast-weight-load path
    moves 4B/cycle; h stays bf16 (moving operand), keeping h exact.
    Gate emission order (i,f) then g then o lets the c-chain elementwise
    overlap the o-gate matmuls, shrinking the serial tail per step.
  - phase 3 is stream-bound at the bf16 matmul roofline (fp8 fails the
    2e-2 accuracy budget - measured 3.3e-2). Its rhs reads h as strided
    views of the bf16 h history (no repack copies), and the first
    `fcnbufs` weight chunks DMA during phase 2 to stay under the HBM
    ceiling (weights-in + logits-out ~= 347 GB/s if unstaggered).

Device layouts (partition dim = feature dim everywhere):
  - gate dim 4H split into 32 slices of 128, permuted [i f o g] so one
    sigmoid covers cols 0:384 and one tanh covers cols 384:512 of the
    per-step [128, 512] gate tile (cols = slice-block * 16 batch).
  - h state history hall[128, t*128 + k*16 + b] (k = H-tile), written
    once per step as one [128, 128] tile; serves as the recurrence rhs
    (contiguous [128,16] slices) and the fcn rhs ([128, (t,b)] strided).
  - vocab projection: out.T tiles [V-tile 128, 384 rows], rows = (t, b).
"""

import numpy as np
import ml_dtypes

import concourse.bacc as bacc
import concourse.mybir as mybir
import concourse.tile as tile
from concourse.bass_utils import run_bass_kernel_spmd

B, T, E, H, V, ENC = 128, 25, 512, 1024, 32000, 400
NCORES = 8
BS = B // NCORES          # 16 batch rows per core
TB = T * BS               # 400 = matmul N for phase 1
ROWS = (T - 1) * BS       # 384 = matmul N for the vocab projection
KT = H // 128             # 8 K-tiles over H
KE = E // 128             # 4 K-tiles over E (word part only)
GS = 4 * H // 128         # 32 gate slices
VT = V // 128             # 250 vocab tiles
NCH = 63                  # fcn weight chunks (512 vocab cols, last 256)
VP = NCH * 512            # 32256 = vocab padded for host layout
VTP = VT                  # 250 tiles computed (no pad tiles)

# torch LSTMCell gate order is [i f g o]; we want [i f o g] so sigmoid is
# one contiguous span. perm_src[j] = source slice for permuted block j.
PERM_SRC = list(range(0, 16)) + list(range(24, 32)) + list(range(16, 24))
# phase-2 matmul emission order: i,f then g (the c-chain inputs), o last.
# i,f,g accumulate in their own PSUM tile that completes 3/4 through the
# wave, so the c-chain elementwise overlaps the o-gate matmuls (PSUM
# dependency tracking is whole-tile, hence the two-tile split).
EMIT_IFG = list(range(0, 16)) + list(range(24, 32))
EMIT_O = list(range(16, 24))

# dtype config: p1/fcn bf16 (accuracy-bound), rec weights fp8 (W_hh only;
# h stays bf16; error 1.1e-2 vs 2e-2 budget, measured in fp-faithful sim).
CFG = {
    "p1": "bf16",
    "rec": "f8",
    "fcn": "bf16",
    "fcnbufs": 10,   # fcn weight chunks prefetched/staged in SBUF
    "heat": 8,       # dense warm-up matmuls at phase-2 end (HAM un-throttle)
}

_F32 = mybir.dt.float32
_DT = {"f32": mybir.dt.float32, "bf16": mybir.dt.bfloat16,
       "f8": mybir.dt.float8e4}
_NPDT = {"f32": np.float32, "bf16": ml_dtypes.bfloat16,
         "f8": ml_dtypes.float8_e4m3}


def build_nc(cfg=CFG):
    AF = mybir.ActivationFunctionType
    p1, rec, fcn = cfg["p1"], cfg["rec"], cfg["fcn"]

    nc = bacc.Bacc()
    xw_d = nc.dram_tensor("xw", [128, KE * TB], _DT[p1], kind="ExternalInput")
    f2t_d = nc.dram_tensor("f2t", [BS, GS * 128], _DT[p1], kind="ExternalInput")
    ind_d = nc.dram_tensor("ind", [BS, TB], _DT[p1], kind="ExternalInput")
    wihw_d = nc.dram_tensor("wihw", [128, KE * 4 * H], _DT[p1], kind="ExternalInput")
    whh_d = nc.dram_tensor("whh", [128, KT * 4 * H], _DT[rec], kind="ExternalInput")
    fcnw_d = nc.dram_tensor("fcnw", [NCH, 128, KT * 512], _DT[fcn], kind="ExternalInput")
    fb_d = nc.dram_tensor("fb", [128, VTP], _F32, kind="ExternalInput")
    out_d = nc.dram_tensor("out", [VTP, 128, ROWS], mybir.dt.float16,
                           kind="ExternalOutput")

    with tile.TileContext(nc) as tc:
        with (
            tc.tile_pool(name="pers", bufs=1) as pers,
            tc.tile_pool(name="psum", bufs=4, space="PSUM") as psum,
            tc.tile_pool(name="elem", bufs=2) as elem,
        ):
            hall = pers.tile([128, T * 128], _DT[p1])
            xw_sb = pers.tile([128, KE * TB], _DT[p1])
            f2t_sb = pers.tile([BS, GS * 128], _DT[p1])
            ind_sb = pers.tile([BS, TB], _DT[p1])
            gxt = pers.tile([128, GS * TB], _DT[p1])
            whh_sb = pers.tile([128, KT * 4 * H], _DT[rec])
            fb_sb = pers.tile([128, VTP], _F32)

            nc.sync.dma_start(xw_sb[:], xw_d[:])
            nc.sync.dma_start(f2t_sb[:], f2t_d[:])
            nc.sync.dma_start(ind_sb[:], ind_d[:])
            nc.sync.dma_start(fb_sb[:], fb_d[:])
            nc.gpsimd.memset(hall[:], 0.0)

            # ------ Phase 1: Gx = [words] @ W_ih_w.T + F (F via K=16 mm) --
            with nc.named_scope("phase1"):
                with tc.tile_pool(name="wihp", bufs=2) as wihp:
                    for q in range(4):
                        wih_sb = wihp.tile([128, KE * 1024], _DT[p1], tag="wih")
                        for k in range(KE):
                            nc.sync.dma_start(
                                wih_sb[:, k * 1024:(k + 1) * 1024],
                                wihw_d[:, k * 4096 + q * 1024:
                                       k * 4096 + q * 1024 + 1024])
                        if q == 0:
                            # W_hh lands behind phase-1's critical inputs so
                            # the first wave isn't starved; needed only at t1
                            for k in range(KT):
                                nc.sync.dma_start(
                                    whh_sb[:, k * 4096:(k + 1) * 4096],
                                    whh_d[:, k * 4096:(k + 1) * 4096])
                        for jj in range(8):
                            j = q * 8 + jj
                            ps = psum.tile([128, TB], _F32, tag="ps",
                                           name="ps1", bufs=4)
                            nc.tensor.matmul(
                                ps[:], f2t_sb[:, j * 128:(j + 1) * 128],
                                ind_sb[:], start=True, stop=False)
                            for k in range(KE):
                                nc.tensor.matmul(
                                    ps[:],
                                    wih_sb[:, k * 1024 + jj * 128:
                                           k * 1024 + jj * 128 + 128],
                                    xw_sb[:, k * TB:(k + 1) * TB],
                                    start=False, stop=(k == KE - 1))
                            nc.scalar.activation(gxt[:, j * TB:(j + 1) * TB],
                                                 ps[:], AF.Identity)

            # ------ Phase 2: LSTM recurrence ------------------------------
            gxt_r = gxt.rearrange("p (j t b) -> p j (t b)", j=GS, t=T, b=BS)
            hall_r = hall.rearrange("p (t k b) -> p k t b", t=T, k=KT, b=BS)

            with nc.named_scope("phase2"):
                c_prev = None
                for t in range(T):
                    if t == 0:
                        gates_src = gxt_r[:, :, 0:BS]  # [128, 32, 16] strided
                        sig_sb = elem.tile([128, 24, BS], _F32, tag="sig", name="sig")
                        nc.scalar.activation(sig_sb[:], gates_src[:, 0:24, :], AF.Sigmoid)
                        tg = elem.tile([128, 8, BS], _F32, tag="tg", name="tg")
                        nc.scalar.activation(tg[:], gates_src[:, 24:32, :], AF.Tanh)
                        sig2 = sig_sb.rearrange("p a b -> p (a b)")
                        tg2 = tg.rearrange("p a b -> p (a b)")
                        cn = elem.tile([128, 128], _F32, tag="c", name="cn")
                        nc.vector.tensor_mul(cn[:], sig2[:, 0:128], tg2[:])
                        thc = elem.tile([128, 128], _F32, tag="thc", name="thc")
                        nc.scalar.activation(thc[:], cn[:], AF.Tanh)
                        nc.vector.tensor_mul(hall[:, t * 128:(t + 1) * 128],
                                             sig2[:, 256:384], thc[:])
                    else:
                        # i,f,g accumulate in ps_a (completes 3/4 into the
                        # wave -> c-chain overlaps the o matmuls in ps_b;
                        # PSUM deps are whole-tile, hence the split)
                        ps_a = psum.tile([128, 16 * BS], _F32, tag="psa",
                                         name="psa", bufs=2)
                        ps_g = psum.tile([128, 8 * BS], _F32, tag="psg",
                                         name="psg", bufs=1)
                        ps_b = psum.tile([128, 8 * BS], _F32, tag="psb",
                                         name="psb", bufs=1)
                        psa3 = ps_a.rearrange("p (j n) -> p j n", n=BS)
                        psg3 = ps_g.rearrange("p (j n) -> p j n", n=BS)
                        psb3 = ps_b.rearrange("p (j n) -> p j n", n=BS)
                        for dst, jlist in ((psa3, range(0, 16)),
                                           (psg3, range(24, 32)),
                                           (psb3, range(16, 24))):
                            for jj, j in enumerate(jlist):
                                for k in range(KT):
                                    nc.tensor.matmul(
                                        dst[:, jj, :],
                                        whh_sb[:, k * 4096 + j * 128:
                                               k * 4096 + j * 128 + 128],
                                        hall_r[:, k, t - 1, :],
                                        start=(k == 0), stop=(k == KT - 1))
                        gates = elem.tile([128, GS, BS], _F32, tag="g", name="g")
                        g2 = gates.rearrange("p a b -> p (a b)")
                        tcols = slice(t * BS, (t + 1) * BS)
                        nc.vector.tensor_add(gates[:, 0:16, :], psa3[:],
                                             gxt_r[:, 0:16, tcols])
                        nc.vector.tensor_add(gates[:, 24:32, :], psg3[:],
                                             gxt_r[:, 24:32, tcols])
                        sig_if = elem.tile([128, 256], _F32, tag="sif", name="sif")
                        nc.scalar.activation(sig_if[:], g2[:, 0:256], AF.Sigmoid)
                        tg = elem.tile([128, 128], _F32, tag="tg", name="tg")
                        nc.scalar.activation(tg[:], g2[:, 384:512], AF.Tanh)
                        cn = elem.tile([128, 128], _F32, tag="c", name="cn")
                        nc.vector.tensor_mul(cn[:], sig_if[:, 128:256], c_prev[:])
                        t1 = elem.tile([128, 128], _F32, tag="t1", name="t1")
                        nc.vector.tensor_mul(t1[:], sig_if[:, 0:128], tg[:])
                        nc.vector.tensor_add(cn[:], cn[:], t1[:])
                        thc = elem.tile([128, 128], _F32, tag="thc", name="thc")
                        nc.scalar.activation(thc[:], cn[:], AF.Tanh)
                        nc.vector.tensor_add(gates[:, 16:24, :], psb3[:, 0:8, :],
                                             gxt_r[:, 16:24, tcols])
                        sig_o = elem.tile([128, 128], _F32, tag="so", name="so")
                        nc.scalar.activation(sig_o[:], g2[:, 256:384], AF.Sigmoid)
                        nc.vector.tensor_mul(hall[:, t * 128:(t + 1) * 128],
                                             sig_o[:], thc[:])
                        # dense junk matmuls fill the c-chain gap: the PE
                        # stays busy through the elementwise so HAM holds
                        # the clock at 8/8 (they queue before next wave's
                        # h-dependent matmuls and have no consumers)
                        for _ in range(cfg.get("stepheat", 8)):
                            hps = psum.tile([128, TB], _F32, tag="ps",
                                            name="sheat", bufs=4)
                            nc.tensor.matmul(hps[:], whh_sb[:, 0:128],
                                             whh_sb[:, 0:400],
                                             start=True, stop=True)
                    c_prev = cn

                # dense warm-up matmuls: un-throttle the PE clock before the
                # stream-bound phase 3 (phase-2 waves run at low duty cycle)
                for _ in range(cfg.get("heat", 0)):
                    hps = psum.tile([128, TB], _F32, tag="ps",
                                    name="heat", bufs=4)
                    nc.tensor.matmul(hps[:], whh_sb[:, 0:128],
                                     whh_sb[:, 0:400], start=True, stop=True)

            # ------ Phase 3: logits = H @ fcn_W.T + fcn_b -----------------
            with nc.named_scope("phase3"):
                with (
                    tc.tile_pool(name="fcnp", bufs=cfg["fcnbufs"]) as fcnp,
                    tc.tile_pool(name="outp", bufs=4) as outp,
                ):
                    for c in range(NCH):
                        nmi = 4 if c < NCH - 1 else 2   # last chunk: 256 cols
                        wt = fcnp.tile([128, KT * 512], _DT[fcn], tag="fw", name="fw")
                        # mi-major layout: each sub-DMA delivers one complete
                        # vocab tile's weights, unblocking its matmuls early
                        for sub in range(nmi):
                            nc.sync.dma_start(
                                wt[:, sub * 1024:(sub + 1) * 1024],
                                fcnw_d[c, :, sub * 1024:(sub + 1) * 1024])
                        for mi in range(nmi):
                            vt = c * 4 + mi
                            ps = psum.tile([128, ROWS], _F32, tag="ps",
                                           name="ps3", bufs=4)
                            for k in range(KT):
                                nc.tensor.matmul(
                                    ps[:],
                                    wt[:, mi * 1024 + k * 128:
                                       mi * 1024 + k * 128 + 128],
                                    hall_r[:, k, 1:T, :],
                                    start=(k == 0), stop=(k == KT - 1))
                            ot = outp.tile([128, ROWS], mybir.dt.float16,
                                           tag="ot", name="ot")
                            nc.scalar.activation(ot[:], ps[:], AF.Identity,
                                                 bias=fb_sb[:, vt:vt + 1])
                            nc.sync.dma_start(out_d[vt], ot[:])

    nc.finalize()
    return nc


def _prep_shared(W_ih, W_hh, b_ih, b_hh, fcn_W, fcn_b, cfg):
    """Host-side layout transforms (plus the tiny F bias matmul pieces)."""
    perm = np.concatenate([np.arange(s * 128, (s + 1) * 128) for s in PERM_SRC])
    p1np, recnp, fcnnp = _NPDT[cfg["p1"]], _NPDT[cfg["rec"]], _NPDT[cfg["fcn"]]

    Wp = np.asarray(W_ih, np.float32)[perm]              # [4H, E+ENC]
    wihwT = Wp[:, :E].T                                  # [E, 4H]
    wihw = np.ascontiguousarray(
        wihwT.reshape(KE, 128, 4 * H).transpose(1, 0, 2).reshape(128, KE * 4 * H)
    ).astype(p1np)
    wihf = np.ascontiguousarray(Wp[:, E:])               # [4H, ENC] f32, host F
    bsum = (np.asarray(b_ih, np.float32) + np.asarray(b_hh, np.float32))[perm]

    whhT = np.asarray(W_hh, np.float32)[perm].T          # [H, 4H]
    whh = np.ascontiguousarray(
        whhT.reshape(KT, 128, 4 * H).transpose(1, 0, 2).reshape(128, KT * 4 * H)
    ).astype(recnp)

    fw = np.zeros((VP, H), np.float32)
    fw[:V] = np.asarray(fcn_W, np.float32)
    # mi-major: fcnw[c, p, mi*1024 + k*128 + q] = fcn_W[c*512+mi*128+q, k*128+p]
    fcnw = np.ascontiguousarray(
        fw.reshape(NCH, 4, 128, KT, 128).transpose(0, 4, 1, 3, 2)
        .reshape(NCH, 128, KT * 512)).astype(fcnnp)

    fb = np.ascontiguousarray(
        np.asarray(fcn_b, np.float32).reshape(VTP, 128).T)

    ind = np.ascontiguousarray(
        np.tile(np.eye(BS, dtype=np.float32), (1, T))).astype(p1np)

    shared = {"wihw": wihw, "whh": whh, "fcnw": fcnw, "fb": fb, "ind": ind}
    return shared, wihf, bsum


def _prep_core(features, captions, emb_W, wihf, bsum, core, cfg):
    p1np = _NPDT[cfg["p1"]]
    sl = slice(core * BS, (core + 1) * BS)
    feats = np.asarray(features, np.float32)[sl]          # [16, ENC]
    caps = np.asarray(captions)[sl]                       # [16, T]
    embW = np.asarray(emb_W, np.float32)

    words = np.empty((BS, T, E), np.float32)
    words[:, 0, :] = embW[1]
    words[:, 1:, :] = embW[caps[:, :-1]]

    xw = np.ascontiguousarray(
        words.transpose(2, 1, 0).reshape(KE, 128, TB)
        .transpose(1, 0, 2).reshape(128, KE * TB)).astype(p1np)

    f2 = feats @ wihf.T + bsum                            # [16, 4H] f32
    f2t = np.ascontiguousarray(f2).astype(p1np)
    return {"xw": xw, "f2t": f2t}


_BUILT = {}


def kernel(features, captions, emb_W, W_ih, W_hh, b_ih, b_hh,
           enc_W, enc_b, dec_W, dec_b, full_W, full_b, fcn_W, fcn_b,
           _cfg=None, _trace=False):
    cfg = dict(CFG if _cfg is None else _cfg)
    key = (cfg["p1"], cfg["rec"], cfg["fcn"], cfg["fcnbufs"], cfg.get("heat", 0))
    if key not in _BUILT:
        _BUILT[key] = build_nc(cfg)
    nc = _BUILT[key]

    shared, wihf, bsum = _prep_shared(W_ih, W_hh, b_ih, b_hh, fcn_W, fcn_b, cfg)
    in_maps = []
    for c in range(NCORES):
        m = dict(shared)
        m.update(_prep_core(features, captions, emb_W, wihf, bsum, c, cfg))
        in_maps.append(m)

    res = run_bass_kernel_spmd(nc, in_maps, list(range(NCORES)), trace=_trace)

    out = np.empty((B, T - 1, V), np.float32)
    for c in range(NCORES):
        o = res.results[c]["out"][:VT].astype(np.float32)  # [VT, 128, ROWS]
        o = o.transpose(2, 0, 1).reshape(T - 1, BS, V)    # rows=(t,b) -> [24,16,V]
        out[c * BS:(c + 1) * BS] = o.transpose(1, 0, 2)
    kernel._last_result = res
    return out
